# revision 27
# baseline (speedup 1.0000x reference)
"""F1-score (histogram_binning) Trainium2 Bass kernel.

Computes: pred = argmax(y_pred, axis=1); cm = confusion_matrix(y_true, pred);
then the scalar F1 epilogue of the reference.

Strategy (data-parallel over samples, 8 cores), engines balanced:
  - Stream y_pred shard in 1MB blocks [128 part(samples) x G=16 groups x 128].
  - VectorE: row-max reduce; is_ge one-hot (one TT) for DVE_GROUPS groups;
    oh_true = (iota == label) as ONE pair-packed bf16 TT (2x_1P mode).
  - ScalarE: Sign(x - max) for the remaining groups -> (oh_pred - 1) in
    {-1, 0}; exact correction recovered on host from row sums + bincount:
    rowsum = hist_all - 128*hist_act  =>  cm[i,j] += hist_act[i].
  - TensorE: cm_psum += oh_true^T @ oh_pred, 1024x 128-contraction matmuls
    accumulating into one PSUM bank.
  - Host: sum 8 partial [128,128] outputs, apply correction, F1 epilogue.

Measured: ~273 us/core HW exec (memory roofline ~179 us at 358 GB/s/core),
F1 bit-exact vs the jax reference.
"""

import sys

import numpy as np

sys.path.insert(0, "/opt/trn_rl_repo")

import ml_dtypes  # noqa: E402

import concourse.bacc as bacc  # noqa: E402
import concourse.bass as bass  # noqa: E402
import concourse.tile as tile  # noqa: E402
from concourse import mybir  # noqa: E402
from concourse.bass_utils import run_bass_kernel_spmd  # noqa: E402

N_CORES = 8
N_SAMPLES = 1048576
C = 128
EPS = 1e-07
N_PER_CORE = N_SAMPLES // N_CORES  # 131072
P = 128  # partitions
F_PER_PART = N_PER_CORE // P  # 1024 samples per partition
G = 16  # sample-groups per block
N_BLOCKS = F_PER_PART // G  # 128
DVE_GROUPS = 5  # groups whose is_ge runs on DVE; rest use ACT Sign path


def build_program():
    nc = bacc.Bacc("TRN2")

    y_pred = nc.dram_tensor(
        "y_pred", [N_PER_CORE, C], mybir.dt.float32, kind="ExternalInput"
    )
    # aux[p, :2*F_PER_PART] = labels duplicated in adjacent pairs (bf16,
    # enables DVE 2x_1P packed reads); then iota 0..C-1; then a 1.0 column.
    AUXW = 2 * F_PER_PART + C + 1
    aux_bf16 = nc.dram_tensor(
        "aux_bf16", [P, AUXW], mybir.dt.bfloat16, kind="ExternalInput"
    )
    out_t = nc.dram_tensor("out", [C, C], mybir.dt.float32, kind="ExternalOutput")

    # blocks whose oh_true is host-precomputed and streamed from HBM
    pre_blocks = [b for b in range(N_BLOCKS) if b % 8 < 5]
    oh_pre_t = nc.dram_tensor(
        "oh_pre", [P, len(pre_blocks), G, C], mybir.dt.bfloat16, kind="ExternalInput"
    )

    # sample s_local = p * F_PER_PART + b*G + g  (each partition owns
    # F_PER_PART consecutive samples -> fully contiguous per-partition DMA)
    xs = y_pred[:].rearrange("(p b g) c -> p b g c", p=P, b=N_BLOCKS, g=G)

    with tile.TileContext(nc) as tc:
        with (
            tc.tile_pool(name="consts", bufs=1) as consts,
            tc.tile_pool(name="xp", bufs=8) as xp,
            tc.tile_pool(name="ohp", bufs=12) as ohp,
            tc.tile_pool(name="small", bufs=8) as small,
            tc.tile_pool(name="psum", bufs=2, space="PSUM") as psum_pool,
            tc.tile_pool(name="outp", bufs=1) as outp,
        ):
            aux_sb = consts.tile([P, AUXW], mybir.dt.bfloat16)
            nc.gpsimd.dma_start(out=aux_sb, in_=aux_bf16[:])
            iota_off = 2 * F_PER_PART
            iota_sl = aux_sb[:, iota_off : iota_off + C]

            cm_psum = psum_pool.tile([C, C], mybir.dt.float32)

            # 4D pair-packed APs (innermost [1,2] bf16 -> DVE 2x_1P mode):
            # iota viewed [P, G(bcast), 64, 2]
            iota_bc = bass.AP(
                tensor=iota_sl.tensor,
                offset=iota_sl.offset,
                ap=[[AUXW, P], [0, G], [2, 64], [1, 2]],
            )

            for b in range(N_BLOCKS):
                x_t = xp.tile([P, G, C], mybir.dt.float32)
                nc.sync.dma_start(out=x_t, in_=xs[:, b])

                rowmax = small.tile([P, G], mybir.dt.float32)
                nc.vector.tensor_reduce(
                    out=rowmax,
                    in_=x_t,
                    axis=mybir.AxisListType.X,
                    op=mybir.AluOpType.max,
                )
                negmax = small.tile([P, G], mybir.dt.float32, tag="negmax")
                nc.vector.tensor_scalar_mul(
                    out=negmax[:, DVE_GROUPS:G],
                    in0=rowmax[:, DVE_GROUPS:G],
                    scalar1=-1.0,
                )

                oh = ohp.tile([P, G, 2 * C], mybir.dt.bfloat16)
                if b in pre_blocks:
                    # oh_true streamed pre-built from HBM (spare bandwidth)
                    nc.sync.dma_start(
                        out=oh[:, :, 0:C], in_=oh_pre_t[:, pre_blocks.index(b)]
                    )
                else:
                    # oh_true = (iota == label), one pair-packed DVE TT
                    labels_pairs = bass.AP(
                        tensor=aux_sb.tensor,
                        offset=aux_sb.offset + b * G * 2,
                        ap=[[AUXW, P], [2, G], [0, 64], [1, 2]],
                    )
                    oh_true_4d = bass.AP(
                        tensor=oh.tensor,
                        offset=oh.offset,
                        ap=[[2 * G * C, P], [2 * C, G], [2, 64], [1, 2]],
                    )
                    nc.vector.tensor_tensor(
                        out=oh_true_4d,
                        in0=iota_bc,
                        in1=labels_pairs,
                        op=mybir.AluOpType.is_equal,
                    )
                # oh_pred = (x >= max) for the DVE share, one TT
                nc.vector.tensor_tensor(
                    out=oh[:, 0:DVE_GROUPS, C : 2 * C],
                    in0=x_t[:, 0:DVE_GROUPS, :],
                    in1=rowmax[:, 0:DVE_GROUPS].to_broadcast([P, DVE_GROUPS, C]),
                    op=mybir.AluOpType.is_ge,
                )
                for g in range(DVE_GROUPS, G):
                    # oh_pred - 1 = Sign(x - max) on ACT ({-1, 0})
                    nc.scalar.activation(
                        out=oh[:, g, C : 2 * C],
                        in_=x_t[:, g, :],
                        func=mybir.ActivationFunctionType.Sign,
                        bias=negmax[:, g : g + 1],
                        scale=1.0,
                    )

                for g in range(G):
                    first = b == 0 and g == 0
                    last = b == N_BLOCKS - 1 and g == G - 1
                    nc.tensor.matmul(
                        cm_psum,
                        lhsT=oh[:, g, 0:C],
                        rhs=oh[:, g, C : 2 * C],
                        start=first,
                        stop=last,
                    )

            res_sb = outp.tile([C, C], mybir.dt.float32)
            nc.vector.tensor_copy(out=res_sb, in_=cm_psum)
            nc.gpsimd.dma_start(out=out_t[:], in_=res_sb)

    nc.finalize()
    return nc


_PROGRAM = None


def _get_program():
    global _PROGRAM
    if _PROGRAM is None:
        _PROGRAM = build_program()
    return _PROGRAM


def _shard_inputs(y_pred, y_true):
    y_pred = np.ascontiguousarray(np.asarray(y_pred), dtype=np.float32)
    y_true = np.asarray(y_true)
    iota = np.broadcast_to(np.arange(C, dtype=np.float32), (P, C))
    ones = np.ones((P, 1), dtype=np.float32)
    in_maps = []
    for c in range(N_CORES):
        sl = slice(c * N_PER_CORE, (c + 1) * N_PER_CORE)
        labels = y_true[sl].astype(np.float32).reshape(P, F_PER_PART)
        labels2 = np.repeat(labels, 2, axis=1)
        aux = np.concatenate([labels2, iota, ones], axis=1).astype(ml_dtypes.bfloat16)
        pre_blocks = [b for b in range(N_BLOCKS) if b % 8 < 5]
        lab3 = labels.reshape(P, N_BLOCKS, G)[:, pre_blocks, :]
        oh_pre = (lab3[..., None] == np.arange(C, dtype=np.float32)).astype(
            ml_dtypes.bfloat16
        )
        in_maps.append({"y_pred": y_pred[sl], "aux_bf16": aux, "oh_pre": oh_pre})
    return in_maps


def _epilogue(cm):
    cm = cm.astype(np.float32)
    TP = np.diagonal(cm)
    FP = (C - 1) * cm[:, 1] + cm[:, 0]
    FN = (C - 1) * cm[1, :] + cm[0, :]
    eps = np.float32(EPS)
    sensitivity = np.mean(TP / (TP + FN + eps), dtype=np.float32)
    precision = np.mean(TP / (TP + FP + eps), dtype=np.float32)
    f1 = np.float32(2.0) * (precision * sensitivity / (precision + sensitivity + eps))
    return np.asarray(f1, dtype=np.float32)


def run_on_device(y_pred, y_true, **kwargs):
    """Run the bass kernel on 8 cores; returns (cm_total, results_obj)."""
    nc = _get_program()
    y_true = np.asarray(y_true)
    in_maps = _shard_inputs(y_pred, y_true)
    res = run_bass_kernel_spmd(nc, in_maps, core_ids=list(range(N_CORES)), **kwargs)
    cm = np.zeros((C, C), dtype=np.float64)
    for c, r in enumerate(res.results):
        out = r["out"].astype(np.float64)
        # ACT-group samples contributed (oh_pred - 1); recover the exact
        # per-true-class count of those samples from row sums + bincount:
        # rowsum = hist_all - 128 * hist_act  =>  hist_act known exactly.
        sl = slice(c * N_PER_CORE, (c + 1) * N_PER_CORE)
        hist_all = np.bincount(np.asarray(y_true[sl]).astype(np.int64), minlength=C)
        hist_act = np.rint((hist_all - out.sum(axis=1)) / C)
        cm += out + hist_act[:, None]
    return cm, res


def kernel(y_pred, y_true):
    cm, _ = run_on_device(y_pred, y_true)
    return _epilogue(cm)


# revision 30
# speedup vs baseline: 1.1257x; 1.1257x over previous
"""F1-score (histogram_binning) Trainium2 Bass kernel.

Computes: pred = argmax(y_pred, axis=1); cm = confusion_matrix(y_true, pred);
then the scalar F1 epilogue of the reference.

Strategy (data-parallel over samples, 8 cores), engines balanced:
  - Stream y_pred shard in 1MB blocks [128 part(samples) x G=16 groups x 128].
  - VectorE: row-max reduce; is_ge one-hot (one TT) for DVE_GROUPS groups;
    oh_true = (iota == label) as ONE pair-packed bf16 TT (2x_1P mode).
  - ScalarE: Sign(x - max) for the remaining groups -> (oh_pred - 1) in
    {-1, 0}; exact correction recovered on host from row sums + bincount:
    rowsum = hist_all - 128*hist_act  =>  cm[i,j] += hist_act[i].
  - TensorE: cm_psum += oh_true^T @ oh_pred, 1024x 128-contraction matmuls
    accumulating into one PSUM bank.
  - Host: sum 8 partial [128,128] outputs, apply correction, F1 epilogue.

Measured: ~273 us/core HW exec (memory roofline ~179 us at 358 GB/s/core),
F1 bit-exact vs the jax reference.
"""

import sys

import numpy as np

sys.path.insert(0, "/opt/trn_rl_repo")

import ml_dtypes  # noqa: E402

import concourse.bacc as bacc  # noqa: E402
import concourse.bass as bass  # noqa: E402
import concourse.tile as tile  # noqa: E402
from concourse import mybir  # noqa: E402
from concourse.bass_utils import run_bass_kernel_spmd  # noqa: E402

N_CORES = 8
N_SAMPLES = 1048576
C = 128
EPS = 1e-07
N_PER_CORE = N_SAMPLES // N_CORES  # 131072
P = 128  # partitions
F_PER_PART = N_PER_CORE // P  # 1024 samples per partition
G = 16  # sample-groups per block
N_BLOCKS = F_PER_PART // G  # 128
DVE_GROUPS = 5  # groups whose is_ge runs on DVE; rest use ACT Sign path


def build_program():
    nc = bacc.Bacc("TRN2")

    y_pred = nc.dram_tensor(
        "y_pred", [N_PER_CORE, C], mybir.dt.float32, kind="ExternalInput"
    )
    # aux[p, :2*F_PER_PART] = labels duplicated in adjacent pairs (bf16,
    # enables DVE 2x_1P packed reads); then iota 0..C-1; then a 1.0 column.
    AUXW = 2 * F_PER_PART + C + 1
    aux_bf16 = nc.dram_tensor(
        "aux_bf16", [P, AUXW], mybir.dt.bfloat16, kind="ExternalInput"
    )
    out_t = nc.dram_tensor("out", [C, C], mybir.dt.float32, kind="ExternalOutput")

    # blocks whose oh_true is host-precomputed and streamed from HBM
    pre_blocks = [b for b in range(N_BLOCKS) if b % 8 < 5]
    oh_pre_t = nc.dram_tensor(
        "oh_pre", [P, len(pre_blocks), G, C], mybir.dt.bfloat16, kind="ExternalInput"
    )

    # sample s_local = p * F_PER_PART + b*G + g  (each partition owns
    # F_PER_PART consecutive samples -> fully contiguous per-partition DMA)
    xs = y_pred[:].rearrange("(p b g) c -> p b g c", p=P, b=N_BLOCKS, g=G)

    with tile.TileContext(nc) as tc:
        with (
            tc.tile_pool(name="consts", bufs=1) as consts,
            tc.tile_pool(name="xp", bufs=8) as xp,
            tc.tile_pool(name="ohp", bufs=12) as ohp,
            tc.tile_pool(name="small", bufs=8) as small,
            tc.tile_pool(name="psum", bufs=2, space="PSUM") as psum_pool,
            tc.tile_pool(name="outp", bufs=1) as outp,
        ):
            aux_sb = consts.tile([P, AUXW], mybir.dt.bfloat16)
            nc.gpsimd.dma_start(out=aux_sb, in_=aux_bf16[:])
            iota_off = 2 * F_PER_PART
            iota_sl = aux_sb[:, iota_off : iota_off + C]

            cm_psum = psum_pool.tile([C, C], mybir.dt.float32)

            # 4D pair-packed APs (innermost [1,2] bf16 -> DVE 2x_1P mode):
            # iota viewed [P, G(bcast), 64, 2]
            iota_bc = bass.AP(
                tensor=iota_sl.tensor,
                offset=iota_sl.offset,
                ap=[[AUXW, P], [0, G], [2, 64], [1, 2]],
            )

            for b in range(N_BLOCKS):
                x_t = xp.tile([P, G, C], mybir.dt.float32)
                nc.sync.dma_start(out=x_t, in_=xs[:, b])

                rowmax = small.tile([P, G], mybir.dt.float32)
                nc.vector.tensor_reduce(
                    out=rowmax,
                    in_=x_t,
                    axis=mybir.AxisListType.X,
                    op=mybir.AluOpType.max,
                )
                negmax = small.tile([P, G], mybir.dt.float32, tag="negmax")
                nc.vector.tensor_scalar_mul(
                    out=negmax[:, DVE_GROUPS:G],
                    in0=rowmax[:, DVE_GROUPS:G],
                    scalar1=-1.0,
                )

                oh_true_t = ohp.tile([P, G, C], mybir.dt.bfloat16, tag="oht")
                oh = ohp.tile([P, G, C], mybir.dt.bfloat16, tag="ohp")
                if b in pre_blocks:
                    # oh_true streamed pre-built from HBM (spare bandwidth),
                    # contiguous 4KB-per-partition destination
                    nc.sync.dma_start(
                        out=oh_true_t, in_=oh_pre_t[:, pre_blocks.index(b)]
                    )
                else:
                    # oh_true = (iota == label), one pair-packed DVE TT
                    labels_pairs = bass.AP(
                        tensor=aux_sb.tensor,
                        offset=aux_sb.offset + b * G * 2,
                        ap=[[AUXW, P], [2, G], [0, 64], [1, 2]],
                    )
                    oh_true_4d = bass.AP(
                        tensor=oh_true_t.tensor,
                        offset=oh_true_t.offset,
                        ap=[[G * C, P], [C, G], [2, 64], [1, 2]],
                    )
                    nc.vector.tensor_tensor(
                        out=oh_true_4d,
                        in0=iota_bc,
                        in1=labels_pairs,
                        op=mybir.AluOpType.is_equal,
                    )
                # oh_pred = (x >= max) for the DVE share, one TT
                nc.vector.tensor_tensor(
                    out=oh[:, 0:DVE_GROUPS, :],
                    in0=x_t[:, 0:DVE_GROUPS, :],
                    in1=rowmax[:, 0:DVE_GROUPS].to_broadcast([P, DVE_GROUPS, C]),
                    op=mybir.AluOpType.is_ge,
                )
                for g in range(DVE_GROUPS, G):
                    # oh_pred - 1 = Sign(x - max) on ACT ({-1, 0})
                    nc.scalar.activation(
                        out=oh[:, g, :],
                        in_=x_t[:, g, :],
                        func=mybir.ActivationFunctionType.Sign,
                        bias=negmax[:, g : g + 1],
                        scale=1.0,
                    )

                for g in range(G):
                    first = b == 0 and g == 0
                    last = b == N_BLOCKS - 1 and g == G - 1
                    nc.tensor.matmul(
                        cm_psum,
                        lhsT=oh_true_t[:, g, :],
                        rhs=oh[:, g, :],
                        start=first,
                        stop=last,
                    )

            res_sb = outp.tile([C, C], mybir.dt.float32)
            nc.vector.tensor_copy(out=res_sb, in_=cm_psum)
            nc.gpsimd.dma_start(out=out_t[:], in_=res_sb)

    nc.finalize()
    return nc


_PROGRAM = None


def _get_program():
    global _PROGRAM
    if _PROGRAM is None:
        _PROGRAM = build_program()
    return _PROGRAM


def _shard_inputs(y_pred, y_true):
    y_pred = np.ascontiguousarray(np.asarray(y_pred), dtype=np.float32)
    y_true = np.asarray(y_true)
    iota = np.broadcast_to(np.arange(C, dtype=np.float32), (P, C))
    ones = np.ones((P, 1), dtype=np.float32)
    in_maps = []
    for c in range(N_CORES):
        sl = slice(c * N_PER_CORE, (c + 1) * N_PER_CORE)
        labels = y_true[sl].astype(np.float32).reshape(P, F_PER_PART)
        labels2 = np.repeat(labels, 2, axis=1)
        aux = np.concatenate([labels2, iota, ones], axis=1).astype(ml_dtypes.bfloat16)
        pre_blocks = [b for b in range(N_BLOCKS) if b % 8 < 5]
        lab3 = labels.reshape(P, N_BLOCKS, G)[:, pre_blocks, :]
        oh_pre = (lab3[..., None] == np.arange(C, dtype=np.float32)).astype(
            ml_dtypes.bfloat16
        )
        in_maps.append({"y_pred": y_pred[sl], "aux_bf16": aux, "oh_pre": oh_pre})
    return in_maps


def _epilogue(cm):
    cm = cm.astype(np.float32)
    TP = np.diagonal(cm)
    FP = (C - 1) * cm[:, 1] + cm[:, 0]
    FN = (C - 1) * cm[1, :] + cm[0, :]
    eps = np.float32(EPS)
    sensitivity = np.mean(TP / (TP + FN + eps), dtype=np.float32)
    precision = np.mean(TP / (TP + FP + eps), dtype=np.float32)
    f1 = np.float32(2.0) * (precision * sensitivity / (precision + sensitivity + eps))
    return np.asarray(f1, dtype=np.float32)


def run_on_device(y_pred, y_true, **kwargs):
    """Run the bass kernel on 8 cores; returns (cm_total, results_obj)."""
    nc = _get_program()
    y_true = np.asarray(y_true)
    in_maps = _shard_inputs(y_pred, y_true)
    res = run_bass_kernel_spmd(nc, in_maps, core_ids=list(range(N_CORES)), **kwargs)
    cm = np.zeros((C, C), dtype=np.float64)
    for c, r in enumerate(res.results):
        out = r["out"].astype(np.float64)
        # ACT-group samples contributed (oh_pred - 1); recover the exact
        # per-true-class count of those samples from row sums + bincount:
        # rowsum = hist_all - 128 * hist_act  =>  hist_act known exactly.
        sl = slice(c * N_PER_CORE, (c + 1) * N_PER_CORE)
        hist_all = np.bincount(np.asarray(y_true[sl]).astype(np.int64), minlength=C)
        hist_act = np.rint((hist_all - out.sum(axis=1)) / C)
        cm += out + hist_act[:, None]
    return cm, res


def kernel(y_pred, y_true):
    cm, _ = run_on_device(y_pred, y_true)
    return _epilogue(cm)
